# revision 1
# baseline (speedup 1.0000x reference)
"""MAB qkv attention kernel for Trainium2 (8 NeuronCores, data-parallel over batch).

Math (per batch b):
  Q = query @ Wq.T + bq ; K = key @ Wk.T + bk
  S = (Q @ K.T) * (T/sqrt(512)) ; A = softmax(S, -1)
  out = (A @ value) @ Wo.T + bo            # raw value, V-projection unused

Implementation notes:
  - G-fusion: S = query @ G @ key.T + (Wk.T @ bq) . key  with G = Wq.T @ Wk.
    bk-terms are constant along the softmax axis and cancel exactly.
  - Logit chain (query@G, Qg@key.T) runs in bf16 hi/lo split precision
    (3 matmuls) because the softmax is near-one-hot (T=100) and tf32-grade
    rounding there moves the output by ~3e-2.
  - P@value and @Wo.T chains run in float32r (1 cyc/row, tf32-grade - plenty).
  - Softmax per 128-row strip: fused PSUM-evict + chained row-max
    (tensor_tensor_reduce), ACT exp with per-partition bias/scale and
    accumulated row-sums; normalization deferred to the output eviction.
"""
import os
import sys

sys.path.insert(0, "/opt/trn_rl_repo")
import numpy as np

B, NQ, NK, D = 16, 2048, 2048, 512
NCORES = 8
BLOC = B // NCORES
P = 128
CO = D // P          # 4 contraction chunks
GW = 512             # i-group width
NG = NQ // GW        # 4 groups
JT = NK // P         # 16 key tiles
JB = NK // 512       # 4 key blocks
ISCALE = 1.0 / float(np.sqrt(np.float32(D)))

_CACHE = {}


def _build():
    import concourse.mybir as mybir
    import concourse.tile as tile
    from concourse import bacc
    from concourse.masks import make_identity

    f32 = mybir.dt.float32
    f32r = mybir.dt.float32r
    bf16 = mybir.dt.bfloat16
    AF = mybir.ActivationFunctionType
    OP = mybir.AluOpType

    nc = bacc.Bacc(None, target_bir_lowering=False)
    q_d = nc.dram_tensor("query", [BLOC, NQ, D], f32, kind="ExternalInput")
    k_d = nc.dram_tensor("key", [BLOC, NK, D], f32, kind="ExternalInput")
    v_d = nc.dram_tensor("value", [BLOC, NK, D], f32, kind="ExternalInput")
    wq_d = nc.dram_tensor("Wq", [D, D], f32, kind="ExternalInput")
    wk_d = nc.dram_tensor("Wk", [D, D], f32, kind="ExternalInput")
    wo_d = nc.dram_tensor("Wo", [D, D], f32, kind="ExternalInput")
    bq_d = nc.dram_tensor("bq", [D], f32, kind="ExternalInput")
    bo_d = nc.dram_tensor("bo", [D], f32, kind="ExternalInput")
    t_d = nc.dram_tensor("T", [1], f32, kind="ExternalInput")
    o_d = nc.dram_tensor("out", [BLOC, NQ, D], f32, kind="ExternalOutput")

    with tile.TileContext(nc) as tc:
        with (
            tc.tile_pool(name="const", bufs=1) as const,
            tc.tile_pool(name="inp", bufs=2) as inp,
            tc.tile_pool(name="big", bufs=1) as big,
            tc.tile_pool(name="grp1", bufs=1) as grp1,
            tc.tile_pool(name="grp2", bufs=2) as grp2,
            tc.tile_pool(name="pstr", bufs=4) as pstr,
            tc.tile_pool(name="ptp", bufs=3) as ptp,
            tc.tile_pool(name="small", bufs=4) as small,
            tc.tile_pool(name="psS", bufs=2, space="PSUM") as psS,
            tc.tile_pool(name="psO", bufs=1, space="PSUM") as psO,
            tc.tile_pool(name="psT", bufs=2, space="PSUM") as psT,
        ):
            # ---------------- constants ----------------
            id32 = const.tile([P, P], f32)
            make_identity(nc, id32)
            id32r = const.tile([P, P], f32r)
            nc.vector.tensor_copy(id32r[:], id32[:])
            ones1 = const.tile([1, P], f32)
            nc.vector.memset(ones1[:], 1.0)

            wq_sb = inp.tile([P, CO, D], f32, tag="in")
            nc.sync.dma_start(wq_sb[:], wq_d.rearrange("(o p) c -> p o c", p=P))
            wk_sb = inp.tile([P, CO, D], f32, tag="in")
            nc.sync.dma_start(wk_sb[:], wk_d.rearrange("(o p) c -> p o c", p=P))
            wo_sb = inp.tile([P, CO, D], f32, tag="in")
            nc.sync.dma_start(wo_sb[:], wo_d.rearrange("(o p) c -> p o c", p=P))
            bq_sb = const.tile([P, CO], f32)
            nc.sync.dma_start(bq_sb[:], bq_d.rearrange("(o p) -> p o", p=P))
            bo_row = const.tile([1, D], f32)
            nc.sync.dma_start(bo_row[:], bo_d.rearrange("(a e) -> a e", a=1))
            t_row = const.tile([1, 1], f32)
            nc.sync.dma_start(t_row[:], t_d.rearrange("(a e) -> a e", a=1))

            # G = Wq.T @ Wk, split to bf16 hi/lo
            g_hi = const.tile([P, CO, D], bf16)
            g_lo = const.tile([P, CO, D], bf16)
            for ct in range(CO):
                g_ps = psT.tile([P, 512], f32, tag="t")
                for dd in range(CO):
                    nc.tensor.matmul(
                        g_ps[:], wq_sb[:, dd, ct * P:(ct + 1) * P], wk_sb[:, dd, :],
                        start=(dd == 0), stop=(dd == CO - 1))
                nc.scalar.activation(g_hi[:, ct, :], g_ps[:], AF.Copy)
                nc.vector.tensor_sub(g_lo[:, ct, :], g_ps[:], g_hi[:, ct, :])

            # WoT[d, e] (float32r) via PE transpose of Wo
            wot = const.tile([P, CO, D], f32r)
            for dt in range(CO):
                t_ps = psT.tile([P, 512], f32, tag="t")
                for eo in range(CO):
                    nc.tensor.transpose(
                        t_ps[:, eo * P:(eo + 1) * P],
                        wo_sb[:, eo, dt * P:(dt + 1) * P], id32)
                nc.vector.tensor_copy(wot[:, dt, :], t_ps[:])

            # u = Wk.T @ bq  -> [c', 1] per chunk; added to Qg rows
            u_sb = const.tile([P, CO], f32)
            for ct in range(CO):
                u_ps = psT.tile([P, 512], f32, tag="t")
                for dd in range(CO):
                    nc.tensor.matmul(
                        u_ps[:, 0:1], wk_sb[:, dd, ct * P:(ct + 1) * P],
                        bq_sb[:, dd:dd + 1],
                        start=(dd == 0), stop=(dd == CO - 1))
                nc.vector.tensor_copy(u_sb[:, ct:ct + 1], u_ps[:, 0:1])

            # bo broadcast to [128, D]; T broadcast to [128, 1] scale
            bo_bc = const.tile([P, D], f32)
            b_ps = psT.tile([P, 512], f32, tag="t")
            nc.tensor.matmul(b_ps[:], ones1[:], bo_row[:], start=True, stop=True)
            nc.vector.tensor_copy(bo_bc[:], b_ps[:])
            t_ps2 = psT.tile([P, 512], f32, tag="t")
            nc.tensor.matmul(t_ps2[:, 0:1], ones1[:], t_row[:], start=True, stop=True)
            scl = const.tile([P, 1], f32)
            nscl = const.tile([P, 1], f32)
            nc.vector.tensor_scalar_mul(scl[:], t_ps2[:, 0:1], ISCALE)
            nc.vector.tensor_scalar_mul(nscl[:], t_ps2[:, 0:1], -ISCALE)

            # ---------------- per batch ----------------
            for b in range(BLOC):
                # keyT split to bf16 hi/lo: [c_in 128, cc 4, j 2048]
                kt_hi = big.tile([P, CO, NK], bf16, tag="kthi")
                kt_lo = big.tile([P, CO, NK], bf16, tag="ktlo")
                for g in range(NG):
                    kin = inp.tile([P, 4, D], f32, tag="in")
                    nc.sync.dma_start(
                        kin[:], k_d[b, g * GW:(g + 1) * GW, :]
                        .rearrange("(no p) c -> p no c", p=P))
                    for no in range(4):
                        t_ps = psT.tile([P, 512], f32, tag="t")
                        for cc in range(CO):
                            nc.tensor.transpose(
                                t_ps[:, cc * P:(cc + 1) * P],
                                kin[:, no, cc * P:(cc + 1) * P], id32)
                        jpos = g * GW + no * P
                        t_r = t_ps[:].rearrange("p (c j) -> p c j", c=CO)
                        nc.scalar.activation(
                            kt_hi[:, :, jpos:jpos + P], t_r, AF.Copy)
                        nc.vector.tensor_sub(
                            kt_lo[:, :, jpos:jpos + P], t_r,
                            kt_hi[:, :, jpos:jpos + P])

                # value load, rounded to float32r via staging copy
                v_r = big.tile([P, JT, D], f32r, tag="v")
                for g in range(NG):
                    vst = inp.tile([P, 4, D], f32, tag="in")
                    nc.sync.dma_start(
                        vst[:], v_d[b, g * GW:(g + 1) * GW, :]
                        .rearrange("(no p) c -> p no c", p=P))
                    nc.vector.tensor_copy(
                        v_r[:, g * 4:(g + 1) * 4, :], vst[:])

                rinv = small.tile([P, JT], f32, tag="rinv")

                for ig in range(NG):
                    # -- queryT (bf16 hi/lo) for this group --
                    qt_hi = grp1.tile([P, CO, GW], bf16, tag="qthi")
                    qt_lo = grp1.tile([P, CO, GW], bf16, tag="qtlo")
                    qin = inp.tile([P, 4, D], f32, tag="in")
                    nc.sync.dma_start(
                        qin[:], q_d[b, ig * GW:(ig + 1) * GW, :]
                        .rearrange("(no p) c -> p no c", p=P))
                    for no in range(4):
                        t_ps = psT.tile([P, 512], f32, tag="t")
                        for cc in range(CO):
                            nc.tensor.transpose(
                                t_ps[:, cc * P:(cc + 1) * P],
                                qin[:, no, cc * P:(cc + 1) * P], id32)
                        t_r = t_ps[:].rearrange("p (c j) -> p c j", c=CO)
                        nc.scalar.activation(
                            qt_hi[:, :, no * P:(no + 1) * P], t_r, AF.Copy)
                        nc.vector.tensor_sub(
                            qt_lo[:, :, no * P:(no + 1) * P], t_r,
                            qt_hi[:, :, no * P:(no + 1) * P])

                    # -- M1': QgT = G.T-chunks @ queryT + u, bf16 hi/lo --
                    qg_hi = grp2.tile([P, CO, GW], bf16, tag="qghi")
                    qg_lo = grp2.tile([P, CO, GW], bf16, tag="qglo")
                    for ct in range(CO):
                        qg_ps = psT.tile([P, 512], f32, tag="t")
                        mmidx = 0
                        for gm, qm in ((g_hi, qt_hi), (g_hi, qt_lo), (g_lo, qt_hi)):
                            for cc in range(CO):
                                nc.tensor.matmul(
                                    qg_ps[:], gm[:, cc, ct * P:(ct + 1) * P],
                                    qm[:, cc, :],
                                    start=(mmidx == 0), stop=(mmidx == 11))
                                mmidx += 1
                        nc.scalar.activation(
                            qg_hi[:, ct, :], qg_ps[:], AF.Identity,
                            bias=u_sb[:, ct:ct + 1])
                        nc.vector.scalar_tensor_tensor(
                            qg_lo[:, ct, :], qg_ps[:], u_sb[:, ct:ct + 1],
                            qg_hi[:, ct, :], op0=OP.add, op1=OP.subtract)

                    # -- M2 + softmax per 128-row strip --
                    pstrips = []
                    for s in range(4):
                        strip = ig * 4 + s
                        p_strip = pstr.tile([P, NK], f32r, tag="p")
                        pstrips.append(p_strip)
                        s_sb = grp2.tile([P, NK], f32, tag="ssb")
                        mx = small.tile([P, 1], f32, tag="mx")
                        ss = small.tile([P, JB], f32, tag="ss")
                        for jb in range(JB):
                            s_ps = psS.tile([P, 512], f32, tag="s")
                            mmidx = 0
                            for qm, km in ((qg_hi, kt_hi), (qg_hi, kt_lo),
                                           (qg_lo, kt_hi)):
                                for ct in range(CO):
                                    nc.tensor.matmul(
                                        s_ps[:],
                                        qm[:, ct, s * P:(s + 1) * P],
                                        km[:, ct, jb * 512:(jb + 1) * 512],
                                        start=(mmidx == 0), stop=(mmidx == 11))
                                    mmidx += 1
                            nc.scalar.activation(
                                s_sb[:, jb * 512:(jb + 1) * 512], s_ps[:],
                                AF.Copy)
                        nc.vector.reduce_max(
                            mx[:, 0:1], s_sb[:],
                            axis=mybir.AxisListType.X)
                        ebias = small.tile([P, 1], f32, tag="eb")
                        nc.vector.tensor_mul(ebias[:], mx[:, 0:1], nscl[:])
                        for jb in range(JB):
                            nc.scalar.activation(
                                p_strip[:, jb * 512:(jb + 1) * 512],
                                s_sb[:, jb * 512:(jb + 1) * 512],
                                AF.Exp, bias=ebias[:, 0:1], scale=scl[:, 0:1],
                                accum_out=ss[:, jb:jb + 1])
                        rt = small.tile([P, 1], f32, tag="rt")
                        nc.vector.tensor_add(rt[:], ss[:, 0:1], ss[:, 1:2])
                        nc.vector.tensor_add(rt[:], rt[:], ss[:, 2:3])
                        nc.vector.tensor_add(rt[:], rt[:], ss[:, 3:4])
                        nc.vector.reciprocal(rinv[:, strip:strip + 1], rt[:])

                    # -- M3: O^T accum over j: lhsT=value, rhs=P^T --
                    o_ps = psO.tile([P, 4 * 512], f32, tag="o")
                    for jt in range(JT):
                        t_ps = psT.tile([P, 512], f32, tag="t")
                        tr = t_ps[:].bitcast(f32r)
                        for s in range(4):
                            nc.tensor.transpose(
                                tr[:, s * P:(s + 1) * P],
                                pstrips[s][:, jt * P:(jt + 1) * P], id32r)
                        pt_sb = ptp.tile([P, 512], f32r, tag="pt")
                        nc.vector.tensor_copy(pt_sb[:], tr)
                        for dt in range(CO):
                            nc.tensor.matmul(
                                o_ps[:, dt * 512:(dt + 1) * 512],
                                v_r[:, jt, dt * P:(dt + 1) * P], pt_sb[:],
                                start=(jt == 0), stop=(jt == JT - 1))
                    ot = grp1.tile([P, CO, GW], f32r, tag="ot")
                    nc.scalar.activation(
                        ot[:], o_ps[:].rearrange("p (d i) -> p d i", d=CO),
                        AF.Copy)

                    # -- M4: out = rinv * (O^T.T @ WoT) + bo --
                    for s in range(4):
                        strip = ig * 4 + s
                        y_ps = psT.tile([P, 512], f32, tag="t")
                        for dt in range(CO):
                            nc.tensor.matmul(
                                y_ps[:], ot[:, dt, s * P:(s + 1) * P],
                                wot[:, dt, :],
                                start=(dt == 0), stop=(dt == CO - 1))
                        y_sb = grp2.tile([P, D], f32, tag="y")
                        nc.scalar.mul(y_sb[:], y_ps[:], rinv[:, strip:strip + 1])
                        nc.vector.tensor_add(y_sb[:], y_sb[:], bo_bc[:])
                        nc.sync.dma_start(
                            o_d[b, strip * P:(strip + 1) * P, :], y_sb[:])

    nc.compile()
    return nc


def _get_nc():
    if "nc" not in _CACHE:
        _CACHE["nc"] = _build()
    return _CACHE["nc"]


def kernel(**inputs):
    from concourse.bass_utils import run_bass_kernel_spmd

    nc = _get_nc()
    f = lambda x: np.ascontiguousarray(np.asarray(x, dtype=np.float32))
    in_maps = []
    for c in range(NCORES):
        sl = slice(c * BLOC, (c + 1) * BLOC)
        in_maps.append({
            "query": f(inputs["query"][sl]),
            "key": f(inputs["key"][sl]),
            "value": f(inputs["value"][sl]),
            "Wq": f(inputs["Wq"]),
            "Wk": f(inputs["Wk"]),
            "Wo": f(inputs["Wo"]),
            "bq": f(inputs["bq"]),
            "bo": f(inputs["bo"]),
            "T": f(inputs["T"]),
        })
    res = run_bass_kernel_spmd(
        nc, in_maps, list(range(NCORES)),
        trace=bool(int(os.environ.get("KERNEL_TRACE", "0"))))
    _CACHE["last_results"] = res
    out = np.concatenate([r["out"] for r in res.results], axis=0)
    return out.astype(np.float32)



# revision 47
# speedup vs baseline: 1.0577x; 1.0577x over previous
"""MAB qkv attention kernel for Trainium2 (8 NeuronCores, data-parallel over batch).

Math (per batch b):
  Q = query @ Wq.T + bq ; K = key @ Wk.T + bk
  S = (Q @ K.T) * (T/sqrt(512)) ; A = softmax(S, -1)
  out = (A @ value) @ Wo.T + bo            # raw value, V-projection unused

Implementation notes:
  - G-fusion: S = query @ G @ key.T + u . key  with G = Wq.T @ Wk and
    u = Wk.T @ bq. bk-terms are constant along the softmax axis and cancel.
    The u term is folded into the Qg PSUM accumulation as a rank-1 matmul.
  - Logit chain (query@G, Qg@key.T) runs as fp16 hi x hi (1 cyc/row) plus
    both fp16-residual cross terms packed into fp8e4 DoubleRow matmuls
    (0.5 cyc/row, one hi/lo pair per operand slot). The chain computes in
    a x32 domain (G pre-scaled by 32) and each fp8 slot pair carries
    reciprocal power-of-two scales chosen so every operand lands in
    fp8e4's normal range (subnormal floor is 2^-9). Effective logit
    precision ~2^-15; ~2x cheaper on PE than a bf16 3-term scheme.
  - P (softmax probs), value, attn and Wo.T run in fp16 (1 cyc/row).
  - Softmax per 128-row strip: ACT copy eviction + DVE row-max
    (tensor_tensor_reduce hard-crashes real HW, do not use); ACT exp
    applies the runtime T-scale (T/sqrt(D)/32) and per-row bias with
    accumulated row-sums; normalization is deferred to the output
    eviction (fused scalar_tensor_tensor: y = y_ps*rinv + bo).
  - P^T for A@V comes from the xbar DMA transpose (fp16, 16x128 tiles),
    issued on the SP queue ONLY: issuing DmaTransposeAnt from the ACT
    HWDGE queue silently corrupts the output on real HW.
  - Cross-group software pipeline: per group, PE runs
    [M2(ig)] [queryT+M1'(ig+1)] [M3+M4(ig-1)], so softmax tails and
    PSUM evictions always have a full phase of slack.
  - Pool engine (gpsimd, SBUF-only) does the fp8 rescales and value
    rounding; its SWDGE queue carries the key/query input DMAs while
    value/out ride the SP HWDGE queue.
"""
import os
import sys

sys.path.insert(0, "/opt/trn_rl_repo")
import numpy as np

B, NQ, NK, D = 16, 2048, 2048, 512
NCORES = 8
BLOC = B // NCORES
P = 128
CO = D // P          # 4 contraction chunks
GW = 512             # i-group width
NG = NQ // GW        # 4 groups
JT = NK // P         # 16 key tiles
JB = NK // 512       # 4 key blocks
ISCALE = 1.0 / float(np.sqrt(np.float32(D)))

_CACHE = {}


def _build():
    import concourse.mybir as mybir
    import concourse.tile as tile
    from concourse import bacc
    from concourse.masks import make_identity

    f32 = mybir.dt.float32
    f16 = mybir.dt.float16
    f8 = mybir.dt.float8e4
    AF = mybir.ActivationFunctionType
    OP = mybir.AluOpType
    DR = mybir.MatmulPerfMode.DoubleRow

    nc = bacc.Bacc(None, target_bir_lowering=False)
    q_d = nc.dram_tensor("query", [BLOC, NQ, D], f32, kind="ExternalInput")
    k_d = nc.dram_tensor("key", [BLOC, NK, D], f32, kind="ExternalInput")
    v_d = nc.dram_tensor("value", [BLOC, NK, D], f32, kind="ExternalInput")
    wq_d = nc.dram_tensor("Wq", [D, D], f32, kind="ExternalInput")
    wk_d = nc.dram_tensor("Wk", [D, D], f32, kind="ExternalInput")
    wo_d = nc.dram_tensor("Wo", [D, D], f32, kind="ExternalInput")
    bq_d = nc.dram_tensor("bq", [D], f32, kind="ExternalInput")
    bo_d = nc.dram_tensor("bo", [D], f32, kind="ExternalInput")
    t_d = nc.dram_tensor("T", [1], f32, kind="ExternalInput")
    o_d = nc.dram_tensor("out", [BLOC, NQ, D], f32, kind="ExternalOutput")

    with tile.TileContext(nc) as tc:
        with (
            tc.tile_pool(name="const", bufs=1) as const,
            tc.tile_pool(name="psS", bufs=2, space="PSUM") as psS,
            tc.tile_pool(name="psO", bufs=1, space="PSUM") as psO,
            tc.tile_pool(name="psT", bufs=4, space="PSUM") as psT,
        ):
            # ---------------- constants ----------------
            id32 = const.tile([P, P], f32)
            make_identity(nc, id32)
            id16 = const.tile([P, P], f16)
            nc.vector.tensor_copy(id16[:], id32[:])
            ones1 = const.tile([1, P], f32)
            nc.vector.memset(ones1[:], 1.0)

            g_hi = const.tile([P, CO, D], f16)
            g8 = const.tile([P, CO, CO, 2, P], f8)
            wot = const.tile([P, CO, D], f16)
            u_sb = const.tile([P, CO], f32)
            bo_bc = const.tile([P, D], f32)
            scl = const.tile([P, 1], f32)
            nscl = const.tile([P, 1], f32)

            # ---------------- weight setup (scoped staging) -------------
            with tc.tile_pool(name="wstage", bufs=1) as wstage:
                wq_sb = wstage.tile([P, CO, D], f32, tag="wq")
                wq_r = wq_d.rearrange("(o p) c -> p o c", p=P)
                nc.sync.dma_start(wq_sb[:, 0:2, :], wq_r[:, 0:2, :])
                nc.scalar.dma_start(wq_sb[:, 2:4, :], wq_r[:, 2:4, :])
                wk32 = wstage.tile([P, CO, D], f32, tag="wk")
                wk_r = wk_d.rearrange("(o p) c -> p o c", p=P)
                nc.sync.dma_start(wk32[:, 0:2, :], wk_r[:, 0:2, :])
                nc.scalar.dma_start(wk32[:, 2:4, :], wk_r[:, 2:4, :])
                wo_sb = wstage.tile([P, CO, D], f32, tag="wo")
                nc.scalar.dma_start(wo_sb[:], wo_d.rearrange("(o p) c -> p o c", p=P))
                bq_sb = const.tile([P, CO], f32)
                nc.sync.dma_start(bq_sb[:], bq_d.rearrange("(o p) -> p o", p=P))
                bo_row = const.tile([1, D], f32)
                nc.sync.dma_start(bo_row[:], bo_d.rearrange("(a e) -> a e", a=1))
                t_row = const.tile([1, 1], f32)
                nc.sync.dma_start(t_row[:], t_d.rearrange("(a e) -> a e", a=1))

                # wk32 <- 32*Wk in place (x32 logit domain)
                nc.vector.tensor_scalar_mul(wk32[:], wk32[:], 32.0)

                # G' = Wq.T @ (32 Wk): fp16 hi + fp8 [lo*32, hi/32]
                for ct in range(CO):
                    g_ps = psT.tile([P, 512], f32, tag="t")
                    for dd in range(CO):
                        nc.tensor.matmul(
                            g_ps[:], wq_sb[:, dd, ct * P:(ct + 1) * P],
                            wk32[:, dd, :],
                            start=(dd == 0), stop=(dd == CO - 1))
                    nc.scalar.activation(g_hi[:, ct, :], g_ps[:], AF.Copy)
                    gtmp = wstage.tile([P, 512], f32, tag="gtmp")
                    nc.vector.tensor_sub(gtmp[:], g_ps[:], g_hi[:, ct, :])
                    nc.vector.tensor_scalar_mul(
                        g8[:, ct, :, 0, :],
                        gtmp[:].rearrange("p (c j) -> p c j", c=CO), 32.0)
                    nc.vector.tensor_scalar_mul(
                        g8[:, ct, :, 1, :],
                        g_hi[:, ct, :].rearrange("p (c j) -> p c j", c=CO),
                        1.0 / 32.0)

                # WoT[d, e] (fp16) via PE transpose of Wo
                for dt in range(CO):
                    t_ps = psT.tile([P, 512], f32, tag="t")
                    for eo in range(CO):
                        nc.tensor.transpose(
                            t_ps[:, eo * P:(eo + 1) * P],
                            wo_sb[:, eo, dt * P:(dt + 1) * P], id32)
                    nc.scalar.activation(wot[:, dt, :], t_ps[:], AF.Copy)

                # u = (32 Wk).T @ bq -> [P, CO] per-partition (x32 domain)
                for ct in range(CO):
                    u_ps = psT.tile([P, 512], f32, tag="t")
                    for dd in range(CO):
                        nc.tensor.matmul(
                            u_ps[:, 0:1], wk32[:, dd, ct * P:(ct + 1) * P],
                            bq_sb[:, dd:dd + 1],
                            start=(dd == 0), stop=(dd == CO - 1))
                    nc.vector.tensor_copy(u_sb[:, ct:ct + 1], u_ps[:, 0:1])

                # bo broadcast to [128, D]; T -> [128, 1] exp scale (/32)
                b_ps = psT.tile([P, 512], f32, tag="t")
                nc.tensor.matmul(b_ps[:], ones1[:], bo_row[:], start=True, stop=True)
                nc.vector.tensor_copy(bo_bc[:], b_ps[:])
                t_ps2 = psT.tile([P, 512], f32, tag="t")
                nc.tensor.matmul(
                    t_ps2[:, 0:1], ones1[:], t_row[:], start=True, stop=True)
                nc.vector.tensor_scalar_mul(scl[:], t_ps2[:, 0:1], ISCALE / 32.0)
                nc.vector.tensor_scalar_mul(nscl[:], t_ps2[:, 0:1], -ISCALE / 32.0)

            # ---------------- work pools ----------------
            with (
                tc.tile_pool(name="inp", bufs=8) as inp,
                tc.tile_pool(name="big", bufs=1) as big,
                tc.tile_pool(name="grp1", bufs=1) as grp1,
                tc.tile_pool(name="grp2", bufs=2) as grp2,
                tc.tile_pool(name="sstage", bufs=2) as sstage,
                tc.tile_pool(name="pstr", bufs=8) as pstr,
                tc.tile_pool(name="small", bufs=4) as small,
            ):
                def prep_qt(b, ig):
                    """queryT fp16 hi + fp8 [hi/32, lo*32] for group ig."""
                    qt_hi = grp1.tile([P, CO, GW], f16, tag="qthi")
                    qt8 = grp1.tile([P, CO, 2, GW], f8, tag="qt8")
                    for no in range(4):
                        r0 = ig * GW + no * P
                        qin = inp.tile([P, D], f32, tag="in1")
                        nc.sync.dma_start(qin[:], q_d[b, r0:r0 + P, :])
                        t_ps = psT.tile([P, 512], f32, tag="t")
                        for cc in range(CO):
                            nc.tensor.transpose(
                                t_ps[:, cc * P:(cc + 1) * P],
                                qin[:, cc * P:(cc + 1) * P], id32)
                        t_r = t_ps[:].rearrange("p (c j) -> p c j", c=CO)
                        hi_sl = qt_hi[:, :, no * P:(no + 1) * P]
                        nc.scalar.activation(hi_sl, t_r, AF.Copy)
                        tmpa = small.tile([P, CO, P], f32, tag="tmpa")
                        nc.vector.tensor_sub(tmpa[:], t_r, hi_sl)
                        nc.gpsimd.tensor_scalar_mul(
                            qt8[:, :, 1, no * P:(no + 1) * P], tmpa[:], 32.0)
                        nc.gpsimd.tensor_scalar_mul(
                            qt8[:, :, 0, no * P:(no + 1) * P], hi_sl, 1.0 / 32.0)
                    return qt_hi, qt8

                def prep_m1(qt_hi, qt8):
                    """M1': QgT' = G'.T @ queryT + 32u (x32 domain)."""
                    qg_hi = grp2.tile([P, CO, GW], f16, tag="qghi")
                    qg8 = grp2.tile([P, CO, 4, 2, P], f8, tag="qg8")
                    for ct in range(CO):
                        qg_ps = psT.tile([P, 512], f32, tag="t")
                        for cc in range(CO):
                            nc.tensor.matmul(
                                qg_ps[:], g_hi[:, cc, ct * P:(ct + 1) * P],
                                qt_hi[:, cc, :], start=(cc == 0), stop=False)
                        for cc in range(CO):
                            nc.tensor.matmul(
                                qg_ps[:], g8[:, cc, ct, :, :],
                                qt8[:, cc, :, :], start=False,
                                stop=(cc == CO - 1), perf_mode=DR)
                        nc.scalar.activation(
                            qg_hi[:, ct, :], qg_ps[:], AF.Identity,
                            bias=u_sb[:, ct:ct + 1])
                        tmpb = small.tile([P, 512], f32, tag="tmpb")
                        nc.vector.scalar_tensor_tensor(
                            tmpb[:], qg_ps[:], u_sb[:, ct:ct + 1],
                            qg_hi[:, ct, :], op0=OP.add, op1=OP.subtract)
                        nc.gpsimd.tensor_scalar_mul(
                            qg8[:, ct, :, 0, :],
                            tmpb[:].rearrange("p (s j) -> p s j", s=4), 4.0)
                        nc.gpsimd.tensor_scalar_mul(
                            qg8[:, ct, :, 1, :],
                            qg_hi[:, ct, :].rearrange("p (s j) -> p s j", s=4),
                            1.0 / 32.0)
                    return qg_hi, qg8

                def m2_softmax(ig, qg_hi, qg8, kt_hi, kt8, rinv):
                    """S strips + fused evict/max + exp; returns 4 P tiles."""
                    p16s = []
                    for s in range(4):
                        strip = ig * 4 + s
                        p16 = pstr.tile([P, NK], f16, tag="p")
                        p16s.append(p16)
                        s_sb = sstage.tile([P, NK], f32, tag="ssb")
                        mx = small.tile([P, 1], f32, tag="mx")
                        ss = small.tile([P, JB], f32, tag="ss")
                        for jb in range(JB):
                            s_ps = psS.tile([P, 512], f32, tag="s")
                            for ct in range(CO):
                                nc.tensor.matmul(
                                    s_ps[:], qg_hi[:, ct, s * P:(s + 1) * P],
                                    kt_hi[:, ct, jb * 512:(jb + 1) * 512],
                                    start=(ct == 0), stop=False)
                            for ct in range(CO):
                                nc.tensor.matmul(
                                    s_ps[:], qg8[:, ct, s, :, :],
                                    kt8[:, ct, jb, :, :],
                                    start=False, stop=(ct == CO - 1),
                                    perf_mode=DR)
                            nc.scalar.activation(
                                s_sb[:, jb * 512:(jb + 1) * 512], s_ps[:],
                                AF.Copy)
                        nc.vector.reduce_max(
                            mx[:, 0:1], s_sb[:], axis=mybir.AxisListType.X)
                        ebias = small.tile([P, 1], f32, tag="eb")
                        nc.vector.tensor_mul(ebias[:], mx[:, 0:1], nscl[:])
                        for jb in range(JB):
                            nc.scalar.activation(
                                p16[:, jb * 512:(jb + 1) * 512],
                                s_sb[:, jb * 512:(jb + 1) * 512],
                                AF.Exp, bias=ebias[:, 0:1], scale=scl[:, 0:1],
                                accum_out=ss[:, jb:jb + 1])
                        rt = small.tile([P, 1], f32, tag="rt")
                        nc.vector.tensor_add(rt[:], ss[:, 0:1], ss[:, 1:2])
                        nc.vector.tensor_add(rt[:], rt[:], ss[:, 2:3])
                        nc.vector.tensor_add(rt[:], rt[:], ss[:, 3:4])
                        nc.vector.reciprocal(rinv[:, strip:strip + 1], rt[:])
                    return p16s

                def m34_m3(b, ig, p16s, v16):
                    """O^T = value^T @ P^T (accum over j); returns ot tile.

                    P^T comes from the DMA xbar transpose (fp16), not PE."""
                    pt_all = grp1.tile([P, 4, JT, P], f16, tag="pt")
                    ot = grp2.tile([P, CO, GW], f16, tag="ot")
                    o_ps = psO.tile([P, 2 * 512], f32, tag="o")
                    o_sA = psS.tile([P, 512], f32, tag="s")
                    o_sB = psS.tile([P, 512], f32, tag="s")
                    use_dmat = bool(int(os.environ.get("KERNEL_DMAT", "1")))
                    if use_dmat:
                        for s in range(4):
                            nc.sync.dma_start_transpose(
                                pt_all[:, s, :, :], p16s[s][:])
                    for jt in range(JT):
                        if not use_dmat:
                            pt_ps = psT.tile([P, 512], f32, tag="t")
                            ptv = pt_ps[:].bitcast(f16)[:, 0:512]
                            for s in range(4):
                                nc.tensor.transpose(
                                    ptv[:, s * P:(s + 1) * P],
                                    p16s[s][:, jt * P:(jt + 1) * P], id16)
                            nc.scalar.activation(
                                pt_all[:, :, jt, :],
                                ptv.rearrange("p (s j) -> p s j", s=4), AF.Copy)
                        for dt, acc in ((0, o_ps[:, 0:512]),
                                        (1, o_ps[:, 512:1024]),
                                        (2, o_sA[:]), (3, o_sB[:])):
                            nc.tensor.matmul(
                                acc, v16[:, jt, dt * P:(dt + 1) * P],
                                pt_all[:, :, jt, :],
                                start=(jt == 0), stop=(jt == JT - 1))
                    nc.vector.tensor_copy(
                        ot[:, 0:2, :], o_ps[:].rearrange("p (d i) -> p d i", d=2))
                    nc.vector.tensor_copy(ot[:, 2, :], o_sA[:])
                    nc.vector.tensor_copy(ot[:, 3, :], o_sB[:])
                    return ot

                def m34_m4(b, ig, ot, rinv):
                    """out strips: y = rinv * (O^T.T @ WoT) + bo."""
                    for s in range(4):
                        strip = ig * 4 + s
                        y_ps = psT.tile([P, 512], f32, tag="t")
                        for dt in range(CO):
                            nc.tensor.matmul(
                                y_ps[:], ot[:, dt, s * P:(s + 1) * P],
                                wot[:, dt, :],
                                start=(dt == 0), stop=(dt == CO - 1))
                        y_sb = grp2.tile([P, D], f32, tag="y")
                        nc.vector.scalar_tensor_tensor(
                            y_sb[:], y_ps[:], rinv[:, strip:strip + 1],
                            bo_bc[:], op0=OP.mult, op1=OP.add)
                        nc.sync.dma_start(
                            o_d[b, strip * P:(strip + 1) * P, :], y_sb[:])

                def kt_prep(b):
                    kt_hi = big.tile([P, CO, NK], f16, tag="kthi")
                    kt8 = big.tile([P, CO, JB, 2, 512], f8, tag="kt8")
                    for g in range(NG):
                        for no in range(4):
                            jpos = g * GW + no * P
                            kin = inp.tile([P, D], f32, tag="in1")
                            keng = nc.gpsimd if (g * 4 + no) % 2 else nc.sync
                            keng.dma_start(kin[:], k_d[b, jpos:jpos + P, :])
                            t_ps = psT.tile([P, 512], f32, tag="t")
                            for cc in range(CO):
                                nc.tensor.transpose(
                                    t_ps[:, cc * P:(cc + 1) * P],
                                    kin[:, cc * P:(cc + 1) * P], id32)
                            t_r = t_ps[:].rearrange("p (c j) -> p c j", c=CO)
                            hi_sl = kt_hi[:, :, jpos:jpos + P]
                            nc.scalar.activation(hi_sl, t_r, AF.Copy)
                            tmpa = small.tile([P, CO, P], f32, tag="tmpa")
                            nc.vector.tensor_sub(tmpa[:], t_r, hi_sl)
                            joff = no * P
                            nc.gpsimd.tensor_scalar_mul(
                                kt8[:, :, g, 1, joff:joff + P], tmpa[:], 32.0)
                            nc.gpsimd.tensor_scalar_mul(
                                kt8[:, :, g, 0, joff:joff + P], hi_sl, 0.25)
                    return kt_hi, kt8

                def v_load(b):
                    v16 = big.tile([P, JT, D], f16, tag="v")
                    for g in range(NG):
                        for no in range(4):
                            r0 = g * GW + no * P
                            vst = inp.tile([P, D], f32, tag="in1")
                            nc.sync.dma_start(vst[:], v_d[b, r0:r0 + P, :])
                            nc.gpsimd.tensor_copy(v16[:, g * 4 + no, :], vst[:])
                    return v16

                # ---- flattened cross-batch pipeline over all groups ----
                # iter i: M2(g_i) | kt-prep(b+1) at batch edge | prep(g_{i+1})
                #         | M3(g_{i-1}) | v16 swap | M4(g_{i-2})
                GS = [(b, ig) for b in range(BLOC) for ig in range(NG)]
                NI = len(GS)
                kt_cur = kt_prep(0)
                qt = prep_qt(*GS[0])
                qg_by_i = {0: prep_m1(*qt)}
                v_by_b = {0: v_load(0)}
                rinv0 = small.tile([P, JT], f32, tag="rinv")
                rinv_by_b = {0: rinv0}
                kt_by_b = {0: kt_cur}
                p16_by_i = {}
                state_by_i = {}

                for i, (b, ig) in enumerate(GS):
                    qg_hi, qg8 = qg_by_i.pop(i)
                    kt_hi, kt8 = kt_by_b[b]
                    p16_by_i[i] = m2_softmax(
                        ig, qg_hi, qg8, kt_hi, kt8, rinv_by_b[b])
                    if ig == NG - 1 and b + 1 < BLOC:
                        kt_by_b[b + 1] = kt_prep(b + 1)
                        rinv_n = small.tile([P, JT], f32, tag="rinv")
                        rinv_by_b[b + 1] = rinv_n
                    if i + 1 < NI:
                        qt = prep_qt(*GS[i + 1])
                        qg_by_i[i + 1] = prep_m1(*qt)
                    if i >= 1:
                        pb, pig = GS[i - 1]
                        m3_out = m34_m3(pb, pig, p16_by_i.pop(i - 1), v_by_b[pb])
                        state_by_i[i - 1] = m3_out
                        if pig == NG - 1 and pb + 1 < BLOC:
                            v_by_b[pb + 1] = v_load(pb + 1)
                    if i >= 2:
                        pb, pig = GS[i - 2]
                        m34_m4(pb, pig, state_by_i.pop(i - 2), rinv_by_b[pb])
                # epilogue
                pb, pig = GS[NI - 1]
                state_by_i[NI - 1] = m34_m3(pb, pig, p16_by_i.pop(NI - 1),
                                            v_by_b[pb])
                pb, pig = GS[NI - 2]
                m34_m4(pb, pig, state_by_i.pop(NI - 2), rinv_by_b[pb])
                pb, pig = GS[NI - 1]
                m34_m4(pb, pig, state_by_i.pop(NI - 1), rinv_by_b[pb])

    nc.compile()
    return nc


def _get_nc():
    if "nc" not in _CACHE:
        _CACHE["nc"] = _build()
    return _CACHE["nc"]


def kernel(**inputs):
    from concourse.bass_utils import run_bass_kernel_spmd

    nc = _get_nc()
    f = lambda x: np.ascontiguousarray(np.asarray(x, dtype=np.float32))
    in_maps = []
    for c in range(NCORES):
        sl = slice(c * BLOC, (c + 1) * BLOC)
        in_maps.append({
            "query": f(inputs["query"][sl]),
            "key": f(inputs["key"][sl]),
            "value": f(inputs["value"][sl]),
            "Wq": f(inputs["Wq"]),
            "Wk": f(inputs["Wk"]),
            "Wo": f(inputs["Wo"]),
            "bq": f(inputs["bq"]),
            "bo": f(inputs["bo"]),
            "T": f(inputs["T"]),
        })
    res = run_bass_kernel_spmd(
        nc, in_maps, list(range(NCORES)),
        trace=bool(int(os.environ.get("KERNEL_TRACE", "0"))))
    _CACHE["last_results"] = res
    out = np.concatenate([r["out"] for r in res.results], axis=0)
    return out.astype(np.float32)
